# revision 14
# baseline (speedup 1.0000x reference)
"""Mamba block (add+RMSNorm -> in_proj -> causal conv1d -> SSM scan -> out_proj)
on 8 Trainium2 NeuronCores.

Sharding: 8-way tensor-parallel over d_inner (256 channels per core); every
core processes all 4096 tokens (both batches, full L=2048 -- the scan
recurrence stays on-core).  Cross-core communication:
  * two small bf16 AllReduces for the x_proj partial sums (one per batch,
    so the first overlaps with the second batch's compute),
  * one bf16 AllToAll of the gated SSM output so that each core runs
    out_proj for one token quarter with the full d_inner contraction
    (avoiding a 16MB AllReduce after out_proj).
Host code only slices / transposes / concatenates.
"""

import sys

for _p in ("/opt/trn_rl_repo", "/root/.axon_site/_ro/trn_rl_repo"):
    if _p not in sys.path:
        sys.path.insert(0, _p)

import numpy as np
from contextlib import ExitStack

import concourse.bacc as bacc
import concourse.mybir as mybir
import concourse.tile as tile
from concourse.bass_utils import run_bass_kernel_spmd
from concourse.masks import make_identity

F32 = mybir.dt.float32
BF16 = mybir.dt.bfloat16
AF = mybir.ActivationFunctionType
OP = mybir.AluOpType

# problem shapes (hardcoded)
DIM = 1024
D_INNER = 2048
D_STATE = 16
D_CONV = 4
DT_RANK = 64
BATCH = 2
SEQ = 2048
EPS = 1e-5

N_CORES = 8
DG = D_INNER // N_CORES          # 256 channels per core
NDT = DG // 128                  # 2 d-tiles per core
NKT = DIM // 128                 # 8 k-tiles over d_model
QTOK = (BATCH * SEQ) // N_CORES  # 512 tokens output slice per core
GROUPS = [list(range(N_CORES))]

_cache = {}
DEBUG = False


def _build():
    if "nc" in _cache:
        return _cache["nc"]

    nc = bacc.Bacc("TRN2", target_bir_lowering=False, debug=False,
                   num_devices=N_CORES)

    NX = DT_RANK + 2 * D_STATE  # 96
    dram_in = lambda n, s, d=F32: nc.declare_dram_parameter(n, list(s), d, isOutput=False)
    dram_out = lambda n, s, d=F32: nc.declare_dram_parameter(n, list(s), d, isOutput=True)

    # ---- inputs (per-core values, same shapes on every core) ----
    hid_T = dram_in("hid_T", (BATCH, DIM, SEQ))          # replicated
    res_T = dram_in("res_T", (BATCH, DIM, SEQ))          # replicated
    hid_q = dram_in("hid_q", (DIM, QTOK))                # core's token quarter
    res_q_in = dram_in("res_q_in", (DIM, QTOK))
    norm_w = dram_in("norm_w", (DIM, 1))
    norm_b = dram_in("norm_b", (DIM, 1))
    inproj_wT = dram_in("inproj_wT", (DIM, 2 * DG))      # [dm, 256 xi rows + 256 z rows]
    conv_diag = dram_in("conv_diag", (D_CONV * NDT * 128, 128), BF16)  # diag mats
    conv_b = dram_in("conv_b", (DG, 1))
    xproj_wT = dram_in("xproj_wT", (DG, DT_RANK + 2 * D_STATE), BF16)
    dtproj_wT = dram_in("dtproj_wT", (DT_RANK, DG), BF16)
    dtproj_b = dram_in("dtproj_b", (DG, 1))
    A_log_g = dram_in("A_log_g", (DG, D_STATE))
    D_g = dram_in("D_g", (DG, 1))
    outproj_wT = dram_in("outproj_wT", (D_INNER, DIM), BF16)  # replicated

    # ---- outputs ----
    out_q = dram_out("out_q", (QTOK, DIM))               # [tok, d_model]
    res_q = dram_out("res_q", (DIM, QTOK))               # [d_model, tok]
    if DEBUG:
        dbg_x = dram_out("dbg_x", (DIM, SEQ))            # normed x, b=0
        dbg_xi = dram_out("dbg_xi", (DG, SEQ), BF16)     # post-conv silu u, b=0
        dbg_sz = dram_out("dbg_sz", (DG, SEQ), BF16)     # silu(z), b=0
        dbg_xdbl = dram_out("dbg_xdbl", (NX, SEQ), BF16) # post-AR x_dbl, b=0
        dbg_dt = dram_out("dbg_dt", (DG, SEQ), BF16)     # dt, b=0
        dbg_yg = dram_out("dbg_yg", (DG, SEQ), BF16)     # gated y, b=0
        dbg_ccb = dram_out("dbg_ccb", (128, NDT))
        dbg_wdiag = dram_out("dbg_wdiag", (128, D_CONV * NDT * 128), BF16)
        dbg_xipre = dram_out("dbg_xipre", (DG, SEQ), BF16)

    # ---- internal DRAM for collectives ----
    ar_in = [nc.dram_tensor(f"ar_in{b}", [NX, SEQ], BF16) for b in range(BATCH)]
    ar_out = [nc.dram_tensor(f"ar_out{b}", [NX, SEQ], BF16, addr_space="Shared")
              for b in range(BATCH)]
    a2a_in = nc.dram_tensor("a2a_in", [N_CORES, DG, QTOK], BF16)
    a2a_out = nc.dram_tensor("a2a_out", [N_CORES, DG, QTOK], BF16)

    LH = SEQ // 2  # L-half for the norm/in_proj stage

    with tile.TileContext(nc) as tc, ExitStack() as ctx:
        wp = ctx.enter_context(tc.tile_pool(name="weights", bufs=1))

        # resident weights
        w_inproj = wp.tile([128, NKT * 2 * DG], F32)       # 8 ktiles side by side
        nc.sync.dma_start(w_inproj[:].rearrange("p (k m) -> p k m", k=NKT),
                          inproj_wT[:].rearrange("(k p) m -> p k m", p=128))
        w_diag = wp.tile([128, D_CONV * NDT * 128], BF16)
        nc.sync.dma_start(w_diag[:].rearrange("p (j m) -> p j m", j=D_CONV * NDT),
                          conv_diag[:].rearrange("(j p) m -> p j m", p=128))
        w_xproj = wp.tile([128, NDT * NX], BF16)
        nc.sync.dma_start(w_xproj[:].rearrange("p (k m) -> p k m", k=NDT),
                          xproj_wT[:].rearrange("(k p) m -> p k m", p=128))
        w_dtproj = wp.tile([64, DG], BF16)
        nc.sync.dma_start(w_dtproj[:], dtproj_wT[:])
        c_nw = wp.tile([128, NKT], F32)
        nc.sync.dma_start(c_nw[:], norm_w[:].rearrange("(k p) o -> p (k o)", p=128).squeeze(-1) if False else norm_w[:].rearrange("(k p) o -> p k o", p=128).squeeze(-1))
        c_nb = wp.tile([128, NKT], F32)
        nc.sync.dma_start(c_nb[:], norm_b[:].rearrange("(k p) o -> p k o", p=128).squeeze(-1))
        c_cb = wp.tile([128, NDT], F32)
        nc.sync.dma_start(c_cb[:], conv_b[:].rearrange("(k p) o -> p k o", p=128).squeeze(-1))
        c_dtb = wp.tile([128, NDT], F32)
        nc.sync.dma_start(c_dtb[:], dtproj_b[:].rearrange("(k p) o -> p k o", p=128).squeeze(-1))
        c_D = wp.tile([128, NDT], F32)
        nc.sync.dma_start(c_D[:], D_g[:].rearrange("(k p) o -> p k o", p=128).squeeze(-1))
        c_Alog = wp.tile([128, NDT * D_STATE], F32)
        nc.sync.dma_start(c_Alog[:].rearrange("p (k n) -> p k n", k=NDT),
                          A_log_g[:].rearrange("(k p) n -> p k n", p=128))
        ones1 = wp.tile([1, 128], F32)
        nc.vector.memset(ones1[:], 1.0)
        ones128 = wp.tile([128, 1], F32)
        nc.vector.memset(ones128[:], 1.0)
        eps_t = wp.tile([1, 1], F32)
        nc.vector.memset(eps_t[:], EPS)
        iden_bf = wp.tile([128, 128], BF16)
        make_identity(nc, iden_bf[:])

        if DEBUG:
            nc.sync.dma_start(dbg_ccb[:], c_cb[:])
            nc.sync.dma_start(dbg_wdiag[:], w_diag[:])

        # A = -exp(A_log): [128, NDT*16]
        c_A = wp.tile([128, NDT * D_STATE], F32)
        nc.scalar.activation(c_A[:], c_Alog[:], AF.Exp)
        nc.vector.tensor_scalar_mul(c_A[:], c_A[:], -1.0)

        # persistent activations (both batches)
        ap_ = ctx.enter_context(tc.tile_pool(name="acts", bufs=1))
        xi = [[ap_.tile([128, SEQ], BF16, tag=f"xi{b}{d}", name=f"xi{b}{d}") for d in range(NDT)]
              for b in range(BATCH)]
        siluz = [[ap_.tile([128, SEQ], BF16, tag=f"sz{b}{d}", name=f"sz{b}{d}") for d in range(NDT)]
                 for b in range(BATCH)]
        dt_t = [[ap_.tile([128, SEQ], BF16, tag=f"dt{b}{d}", name=f"dt{b}{d}") for d in range(NDT)]
                for b in range(BATCH)]

        # ---------- residual output (core's token quarter) ----------
        with tc.tile_pool(name="resq", bufs=2) as rp:
            for kt in range(NKT):
                th = rp.tile([128, QTOK], F32, tag="th", name="th")
                nc.sync.dma_start(th[:], hid_q[kt * 128:(kt + 1) * 128, :])
                tr = rp.tile([128, QTOK], F32, tag="tr", name="tr")
                nc.sync.dma_start(tr[:], res_q_in[kt * 128:(kt + 1) * 128, :])
                ts_ = rp.tile([128, QTOK], F32, tag="ts", name="ts")
                nc.gpsimd.tensor_add(ts_[:], th[:], tr[:])
                nc.sync.dma_start(res_q[kt * 128:(kt + 1) * 128, :], ts_[:])

        # ---------- phases A-D per batch ----------
        xi_pre = [ap_.tile([128, SEQ], BF16, tag=f"xp{d}", name=f"xp{d}") for d in range(NDT)]

        for b in range(BATCH):
            with tc.tile_pool(name=f"norm{b}", bufs=1) as np_, \
                 tc.tile_pool(name=f"normps{b}", bufs=1, space="PSUM") as nps, \
                 tc.tile_pool(name=f"mmps{b}", bufs=4, space="PSUM") as mps:
                for lh in range(2):
                    sl = slice(lh * LH, (lh + 1) * LH)
                    res_t = [np_.tile([128, LH], F32, tag=f"res{k}", name=f"res{k}") for k in range(NKT)]
                    ssq = nps.tile([1, LH], F32, tag="ssq", name="ssq")
                    for kt in range(NKT):
                        th = np_.tile([128, LH], F32, tag="th", name="th")
                        nc.sync.dma_start(th[:], hid_T[b, kt * 128:(kt + 1) * 128, sl])
                        tr = np_.tile([128, LH], F32, tag="tr", name="tr")
                        nc.sync.dma_start(tr[:], res_T[b, kt * 128:(kt + 1) * 128, sl])
                        nc.gpsimd.tensor_add(res_t[kt][:], th[:], tr[:])
                        sq = np_.tile([128, LH], F32, tag="sq", name="sq")
                        nc.scalar.activation(sq[:], res_t[kt][:], AF.Square)
                        for lc in range(LH // 512):
                            nc.tensor.matmul(ssq[:, lc * 512:(lc + 1) * 512],
                                             ones128[:],
                                             sq[:, lc * 512:(lc + 1) * 512],
                                             start=(kt == 0), stop=(kt == NKT - 1))
                    # rstd = exp(-0.5*ln(mean + eps))  (sqrt/rsqrt not in act tables)
                    lnv = np_.tile([1, LH], F32, tag="lnv", name="lnv")
                    nc.scalar.activation(lnv[:], ssq[:], AF.Ln, bias=eps_t[:],
                                         scale=1.0 / DIM)
                    rstd = np_.tile([1, LH], F32, tag="rstd", name="rstd")
                    nc.scalar.activation(rstd[:], lnv[:], AF.Exp, scale=-0.5)
                    rstd_rep = nps.tile([128, LH], F32, tag="rrep", name="rrep")
                    for lc in range(LH // 512):
                        nc.tensor.matmul(rstd_rep[:, lc * 512:(lc + 1) * 512],
                                         ones1[:], rstd[:, lc * 512:(lc + 1) * 512],
                                         start=True, stop=True)
                    x_t = []
                    for kt in range(NKT):
                        xx = np_.tile([128, LH], F32, tag=f"x{kt}", name=f"x{kt}")
                        nc.vector.scalar_tensor_tensor(
                            xx[:], res_t[kt][:], c_nw[:, kt:kt + 1], rstd_rep[:],
                            OP.mult, OP.mult)
                        nc.vector.tensor_scalar_add(xx[:], xx[:], c_nb[:, kt:kt + 1])
                        if DEBUG and b == 0:
                            nc.sync.dma_start(dbg_x[kt * 128:(kt + 1) * 128, sl], xx[:])
                        x_t.append(xx)
                    # in_proj for this L-half
                    for mt in range(2 * DG // 128):       # 4 m-tiles (2 xi + 2 z)
                        for lc in range(LH // 512):
                            pt = mps.tile([128, 512], F32, tag="mm", name="mm")
                            for kt in range(NKT):
                                nc.tensor.matmul(
                                    pt[:],
                                    w_inproj[:, (kt * 2 * DG) + mt * 128:
                                             (kt * 2 * DG) + (mt + 1) * 128],
                                    x_t[kt][:, lc * 512:(lc + 1) * 512],
                                    start=(kt == 0), stop=(kt == NKT - 1))
                            col = slice(lh * LH + lc * 512, lh * LH + (lc + 1) * 512)
                            if mt < NDT:
                                nc.scalar.activation(xi_pre[mt][:, col], pt[:], AF.Copy)
                            else:
                                nc.scalar.activation(siluz[b][mt - NDT][:, col], pt[:],
                                                     AF.Silu)

            # ---------- conv (diag matmul) + silu, x_proj partial ----------
            with tc.tile_pool(name=f"cps{b}", bufs=4, space="PSUM") as cps, \
                 tc.tile_pool(name=f"cv{b}", bufs=2) as cvp:
                for d in range(NDT):
                    for lc in range(SEQ // 512):
                        pt = cps.tile([128, 512], F32, tag="conv", name="conv")
                        base = lc * 512
                        for j in range(D_CONV):
                            shift = D_CONV - 1 - j       # input col = out col - shift
                            lo, hi = base - shift, base + 512 - shift
                            olo = 0
                            if lo < 0:
                                olo, lo = -lo, 0
                            nc.tensor.matmul(
                                pt[:, olo:512],
                                w_diag[:, (j * NDT + d) * 128:(j * NDT + d + 1) * 128],
                                xi_pre[d][:, lo:hi],
                                start=(j == 0), stop=(j == D_CONV - 1),
                                skip_group_check=True)
                        nc.scalar.activation(xi[b][d][:, base:base + 512], pt[:],
                                             AF.Silu, bias=c_cb[:, d:d + 1])
                if DEBUG and b == 0:
                    for d in range(NDT):
                        nc.sync.dma_start(dbg_xipre[d * 128:(d + 1) * 128, :],
                                          xi_pre[d][:])
                # x_proj partial: [96, SEQ] = xproj_wT.T @ xi
                xdbl_sb = cvp.tile([NX, SEQ], BF16, tag="xdbl", name="xdbl")
                for lc in range(SEQ // 512):
                    pt = cps.tile([NX, 512], F32, tag="xproj", name="xproj")
                    for d in range(NDT):
                        nc.tensor.matmul(pt[:], w_xproj[:, d * NX:(d + 1) * NX],
                                         xi[b][d][:, lc * 512:(lc + 1) * 512],
                                         start=(d == 0), stop=(d == NDT - 1))
                    nc.scalar.activation(xdbl_sb[:, lc * 512:(lc + 1) * 512], pt[:],
                                         AF.Copy)
                if DEBUG and b == 0:
                    for d in range(NDT):
                        nc.sync.dma_start(dbg_xi[d * 128:(d + 1) * 128, :], xi[b][d][:])
                        nc.sync.dma_start(dbg_sz[d * 128:(d + 1) * 128, :], siluz[b][d][:])
                nc.sync.dma_start(ar_in[b][:], xdbl_sb[:])

            nc.gpsimd.collective_compute(
                "AllReduce", OP.add, ins=[ar_in[b][:]], outs=[ar_out[b][:]],
                replica_groups=GROUPS)

        # ---------- post-AR: dt_proj ----------
        du = [[ap_.tile([128, SEQ], BF16, tag=f"du{b}{d}", name=f"du{b}{d}") for d in range(NDT)]
              for b in range(BATCH)]
        for b in range(BATCH):
            with tc.tile_pool(name=f"dt{b}", bufs=2) as dp, \
                 tc.tile_pool(name=f"dtps{b}", bufs=4, space="PSUM") as dps:
                dtlow = dp.tile([DT_RANK, SEQ], BF16, tag="dtlow", name="dtlow")
                nc.sync.dma_start(dtlow[:], ar_out[b][0:DT_RANK, :])
                for d in range(NDT):
                    for lc in range(SEQ // 512):
                        pt = dps.tile([128, 512], F32, tag="dtmm", name="dtmm")
                        nc.tensor.matmul(pt[:], w_dtproj[:, d * 128:(d + 1) * 128],
                                         dtlow[:, lc * 512:(lc + 1) * 512],
                                         start=True, stop=True)
                        et = dp.tile([128, 512], F32, tag="spexp", name="spexp")
                        nc.scalar.activation(et[:], pt[:], AF.Exp,
                                             bias=c_dtb[:, d:d + 1])
                        nc.scalar.activation(dt_t[b][d][:, lc * 512:(lc + 1) * 512],
                                             et[:], AF.Ln, bias=ones128[:, 0:1])
                    nc.vector.tensor_tensor(du[b][d][:], dt_t[b][d][:], xi[b][d][:],
                                            OP.mult)
                    if DEBUG and b == 0:
                        nc.sync.dma_start(dbg_dt[d * 128:(d + 1) * 128, :], dt_t[b][d][:])
                if DEBUG and b == 0:
                    nc.sync.dma_start(dbg_xdbl[:], ar_out[b][:])

        # ---------- scan ----------
        yg = [[ap_.tile([128, SEQ], BF16, tag=f"du{b}{d}", name=f"du{b}{d}") for d in range(NDT)]
              for b in range(BATCH)]
        with tc.tile_pool(name="scanps", bufs=1, space="PSUM") as sps, \
             tc.tile_pool(name="scan", bufs=2) as sp:
            for b in range(BATCH):
                y_acc = [sps.tile([128, SEQ], F32, tag=f"yacc{d}", name=f"yacc{d}") for d in range(NDT)]
                for n in range(D_STATE):
                    b_rep = sp.tile([128, SEQ], BF16, tag="brep", name="brep")
                    nc.sync.dma_start(
                        b_rep[:],
                        ar_out[b][DT_RANK + n:DT_RANK + n + 1, :].to_broadcast((128, SEQ)))
                    c_rep = sp.tile([128, SEQ], BF16, tag="crep", name="crep")
                    nc.sync.dma_start(
                        c_rep[:],
                        ar_out[b][DT_RANK + D_STATE + n:DT_RANK + D_STATE + n + 1, :]
                        .to_broadcast((128, SEQ)))
                    for d in range(NDT):
                        dA = sp.tile([128, SEQ], F32, tag="dA", name="dA")
                        nc.scalar.activation(dA[:], dt_t[b][d][:], AF.Exp,
                                             scale=c_A[:, d * D_STATE + n:
                                                       d * D_STATE + n + 1])
                        dBu = sp.tile([128, SEQ], BF16, tag="dBu", name="dBu")
                        nc.vector.tensor_tensor(dBu[:], du[b][d][:], b_rep[:], OP.mult)
                        h = sp.tile([128, SEQ], BF16, tag="h", name="h")
                        nc.vector.tensor_tensor_scan(h[:], dA[:], dBu[:], 0.0,
                                                     OP.mult, OP.add)
                        hC = sp.tile([128, SEQ], BF16, tag="hC", name="hC")
                        nc.vector.tensor_tensor(hC[:], h[:], c_rep[:], OP.mult)
                        for lc in range(SEQ // 512):
                            nc.tensor.matmul(
                                y_acc[d][:, lc * 512:(lc + 1) * 512], iden_bf[:],
                                hC[:, lc * 512:(lc + 1) * 512],
                                start=(n == 0), stop=(n == D_STATE - 1),
                                skip_group_check=True)
                # drain + gate:  yg = (y + D*u) * silu(z)
                for d in range(NDT):
                    t1 = sp.tile([128, SEQ], BF16, tag="t1", name="t1")
                    nc.vector.scalar_tensor_tensor(t1[:], xi[b][d][:], c_D[:, d:d + 1],
                                                   y_acc[d][:], OP.mult, OP.add)
                    nc.vector.tensor_tensor(yg[b][d][:], t1[:], siluz[b][d][:], OP.mult)
                    if DEBUG and b == 0:
                        nc.sync.dma_start(dbg_yg[d * 128:(d + 1) * 128, :], yg[b][d][:])

        # ---------- stage A2A ----------
        for b in range(BATCH):
            for d in range(NDT):
                for q in range(4):
                    s = b * 4 + q
                    nc.sync.dma_start(
                        a2a_in[s, d * 128:(d + 1) * 128, :],
                        yg[b][d][:, q * QTOK:(q + 1) * QTOK])
        nc.gpsimd.collective_compute("AllToAll", OP.bypass, ins=[a2a_in[:]],
                                     outs=[a2a_out[:]], replica_groups=GROUPS)

        # ---------- out_proj for the core's token quarter ----------
        with tc.tile_pool(name="oproj", bufs=2) as op_, \
             tc.tile_pool(name="ops", bufs=4, space="PSUM") as ops:
            w_outproj = op_.tile([128, (D_INNER // 128) * DIM], BF16, tag="wout", name="wout")
            nc.sync.dma_start(w_outproj[:].rearrange("p (k m) -> p k m", k=D_INNER // 128),
                              outproj_wT[:].rearrange("(k p) m -> p k m", p=128))
            yf = []
            for kt in range(D_INNER // 128):
                t = op_.tile([128, QTOK], BF16, tag=f"yf{kt}", name=f"yf{kt}")
                nc.sync.dma_start(t[:], a2a_out[:].rearrange("s d q -> (s d) q")
                                  [kt * 128:(kt + 1) * 128, :])
                yf.append(t)
            for mt in range(QTOK // 128):
                for nck in range(DIM // 512):
                    pt = ops.tile([128, 512], F32, tag="omm", name="omm")
                    for kt in range(D_INNER // 128):
                        nc.tensor.matmul(
                            pt[:], yf[kt][:, mt * 128:(mt + 1) * 128],
                            w_outproj[:, kt * DIM + nck * 512:
                                      kt * DIM + (nck + 1) * 512],
                            start=(kt == 0), stop=(kt == D_INNER // 128 - 1))
                    ot = op_.tile([128, 512], F32, tag="osb", name="osb")
                    nc.scalar.activation(ot[:], pt[:], AF.Copy)
                    nc.sync.dma_start(
                        out_q[mt * 128:(mt + 1) * 128, nck * 512:(nck + 1) * 512],
                        ot[:])

    nc.compile()
    _cache["nc"] = nc
    return nc


def kernel(hidden_states, residual, norm_weight, norm_bias, in_proj_w, conv_w,
           conv_b, x_proj_w, dt_proj_w, dt_proj_b, A_log, D_param, out_proj_w):
    nc = _build()
    f32 = np.float32
    bf16 = None
    import ml_dtypes
    bf16 = ml_dtypes.bfloat16

    hid_T = np.ascontiguousarray(np.swapaxes(np.asarray(hidden_states, f32), 1, 2))
    res_T = np.ascontiguousarray(np.swapaxes(np.asarray(residual, f32), 1, 2))
    hid_flat = np.asarray(hidden_states, f32).reshape(BATCH * SEQ, DIM)
    res_flat = np.asarray(residual, f32).reshape(BATCH * SEQ, DIM)
    outproj_wT = np.ascontiguousarray(np.asarray(out_proj_w, f32).T).astype(bf16)

    in_maps = []
    for g in range(N_CORES):
        dg = slice(g * DG, (g + 1) * DG)
        w_x = np.asarray(in_proj_w[dg.start:dg.stop], f32)           # xi rows
        w_z = np.asarray(in_proj_w[D_INNER + dg.start:D_INNER + dg.stop], f32)
        inproj_wT = np.ascontiguousarray(np.concatenate([w_x, w_z], 0).T)
        cw = np.asarray(conv_w[dg], f32)                             # (256, 4)
        diag = np.zeros((D_CONV, NDT, 128, 128), f32)
        for j in range(D_CONV):
            for d in range(NDT):
                np.fill_diagonal(diag[j, d], cw[d * 128:(d + 1) * 128, j])
        qs = slice(g * QTOK, (g + 1) * QTOK)
        in_maps.append({
            "hid_T": hid_T,
            "res_T": res_T,
            "hid_q": np.ascontiguousarray(hid_flat[qs].T),
            "res_q_in": np.ascontiguousarray(res_flat[qs].T),
            "norm_w": np.asarray(norm_weight, f32).reshape(DIM, 1),
            "norm_b": np.asarray(norm_bias, f32).reshape(DIM, 1),
            "inproj_wT": inproj_wT,
            "conv_diag": diag.reshape(D_CONV * NDT * 128, 128).astype(bf16),
            "conv_b": np.asarray(conv_b[dg], f32).reshape(DG, 1),
            "xproj_wT": np.ascontiguousarray(np.asarray(x_proj_w, f32)[:, dg].T).astype(bf16),
            "dtproj_wT": np.ascontiguousarray(np.asarray(dt_proj_w, f32)[dg].T).astype(bf16),
            "dtproj_b": np.asarray(dt_proj_b[dg], f32).reshape(DG, 1),
            "A_log_g": np.asarray(A_log[dg], f32),
            "D_g": np.asarray(D_param[dg], f32).reshape(DG, 1),
            "outproj_wT": outproj_wT,
        })

    res = run_bass_kernel_spmd(nc, in_maps, list(range(N_CORES)))

    out_flat = np.empty((BATCH * SEQ, DIM), f32)
    resid_flat = np.empty((BATCH * SEQ, DIM), f32)
    for g in range(N_CORES):
        qs = slice(g * QTOK, (g + 1) * QTOK)
        out_flat[qs] = res.results[g]["out_q"]
        resid_flat[qs] = res.results[g]["res_q"].T
    return (out_flat.reshape(BATCH, SEQ, DIM),
            resid_flat.reshape(BATCH, SEQ, DIM))


# revision 20
# speedup vs baseline: 6233.0931x; 6233.0931x over previous
"""Mamba block (add+RMSNorm -> in_proj -> causal conv1d -> SSM scan -> out_proj)
on 8 Trainium2 NeuronCores.

Sharding: 8-way tensor-parallel over d_inner (256 channels per core); every
core processes all 4096 tokens (both batches, full L=2048 -- the scan
recurrence stays on-core).  Cross-core communication:
  * two small bf16 AllReduces for the x_proj partial sums (one per batch,
    so the first overlaps with the second batch's compute),
  * one bf16 AllToAll of the gated SSM output so that each core runs
    out_proj for one token quarter with the full d_inner contraction
    (avoiding a 16MB AllReduce after out_proj).
Host code only slices / transposes / concatenates.
"""

import sys

for _p in ("/opt/trn_rl_repo", "/root/.axon_site/_ro/trn_rl_repo"):
    if _p not in sys.path:
        sys.path.insert(0, _p)

import numpy as np
from contextlib import ExitStack

import concourse.bacc as bacc
import concourse.mybir as mybir
import concourse.tile as tile
from concourse.bass_utils import run_bass_kernel_spmd
from concourse.masks import make_identity

F32 = mybir.dt.float32
BF16 = mybir.dt.bfloat16
AF = mybir.ActivationFunctionType
OP = mybir.AluOpType

# problem shapes (hardcoded)
DIM = 1024
D_INNER = 2048
D_STATE = 16
D_CONV = 4
DT_RANK = 64
BATCH = 2
SEQ = 2048
EPS = 1e-5

N_CORES = 8
DG = D_INNER // N_CORES          # 256 channels per core
NDT = DG // 128                  # 2 d-tiles per core
NKT = DIM // 128                 # 8 k-tiles over d_model
QTOK = (BATCH * SEQ) // N_CORES  # 512 tokens output slice per core
GROUPS = [list(range(N_CORES))]

_cache = {}
DEBUG = False
SIM_NO_COLLECTIVES = False


def _build():
    if "nc" in _cache:
        return _cache["nc"]

    nc = bacc.Bacc("TRN2", target_bir_lowering=False, debug=False,
                   num_devices=N_CORES)

    NX = DT_RANK + 2 * D_STATE  # 96
    dram_in = lambda n, s, d=F32: nc.declare_dram_parameter(n, list(s), d, isOutput=False)
    dram_out = lambda n, s, d=F32: nc.declare_dram_parameter(n, list(s), d, isOutput=True)

    # ---- inputs (per-core values, same shapes on every core) ----
    hid_T = dram_in("hid_T", (BATCH, DIM, SEQ))          # replicated
    res_T = dram_in("res_T", (BATCH, DIM, SEQ))          # replicated
    hid_q = dram_in("hid_q", (DIM, QTOK))                # core's token quarter
    res_q_in = dram_in("res_q_in", (DIM, QTOK))
    norm_w = dram_in("norm_w", (DIM, 1))
    norm_b = dram_in("norm_b", (DIM, 1), BF16)
    inproj_wT = dram_in("inproj_wT", (DIM, 2 * DG), BF16)  # [dm, 256 xi + 256 z rows]
    conv_diag = dram_in("conv_diag", (D_CONV * NDT * 128, 128), BF16)  # diag mats
    conv_b = dram_in("conv_b", (DG, 1))
    xproj_wT = dram_in("xproj_wT", (DG, DT_RANK + 2 * D_STATE), BF16)
    dtproj_wT = dram_in("dtproj_wT", (DT_RANK, DG), BF16)
    dtproj_b = dram_in("dtproj_b", (DG, 1))
    A_log_g = dram_in("A_log_g", (DG, D_STATE))
    D_g = dram_in("D_g", (DG, 1))
    outproj_wT = dram_in("outproj_wT", (D_INNER, DIM), BF16)  # replicated

    # ---- outputs ----
    out_q = dram_out("out_q", (QTOK, DIM))               # [tok, d_model]
    res_q = dram_out("res_q", (DIM, QTOK))               # [d_model, tok]
    if DEBUG:
        dbg_x = dram_out("dbg_x", (DIM, SEQ))            # normed x, b=0
        dbg_xi = dram_out("dbg_xi", (DG, SEQ), BF16)     # post-conv silu u, b=0
        dbg_sz = dram_out("dbg_sz", (DG, SEQ), BF16)     # silu(z), b=0
        dbg_xdbl = dram_out("dbg_xdbl", (NX, SEQ), BF16) # post-AR x_dbl, b=0
        dbg_dt = dram_out("dbg_dt", (DG, SEQ), BF16)     # dt, b=0
        dbg_yg = dram_out("dbg_yg", (DG, SEQ), BF16)     # gated y, b=0
        dbg_ccb = dram_out("dbg_ccb", (128, NDT))
        dbg_wdiag = dram_out("dbg_wdiag", (128, D_CONV * NDT * 128), BF16)
        dbg_xipre = dram_out("dbg_xipre", (DG, SEQ), BF16)

    # ---- internal DRAM for collectives ----
    ar_in = [nc.dram_tensor(f"ar_in{b}", [NX, SEQ], BF16) for b in range(BATCH)]
    ar_out = [nc.dram_tensor(f"ar_out{b}", [NX, SEQ], BF16, addr_space="Shared")
              for b in range(BATCH)]
    a2a_in = nc.dram_tensor("a2a_in", [N_CORES, DG, QTOK], BF16)
    a2a_out = nc.dram_tensor("a2a_out", [N_CORES, DG, QTOK], BF16)

    LH = SEQ // 2  # L-half for the norm/in_proj stage

    with tile.TileContext(nc) as tc, ExitStack() as ctx:
        wp = ctx.enter_context(tc.tile_pool(name="weights", bufs=1))

        # resident weights
        w_inproj = wp.tile([128, NKT * 2 * DG], BF16)      # 8 ktiles side by side
        nc.sync.dma_start(w_inproj[:].rearrange("p (k m) -> p k m", k=NKT),
                          inproj_wT[:].rearrange("(k p) m -> p k m", p=128))
        w_diag = wp.tile([128, D_CONV * NDT * 128], BF16)
        nc.sync.dma_start(w_diag[:].rearrange("p (j m) -> p j m", j=D_CONV * NDT),
                          conv_diag[:].rearrange("(j p) m -> p j m", p=128))
        w_xproj = wp.tile([128, NDT * NX], BF16)
        nc.sync.dma_start(w_xproj[:].rearrange("p (k m) -> p k m", k=NDT),
                          xproj_wT[:].rearrange("(k p) m -> p k m", p=128))
        w_dtproj = wp.tile([64, DG], BF16)
        nc.sync.dma_start(w_dtproj[:], dtproj_wT[:])
        c_nw = wp.tile([128, NKT], F32)
        nc.sync.dma_start(c_nw[:], norm_w[:].rearrange("(k p) o -> p (k o)", p=128).squeeze(-1) if False else norm_w[:].rearrange("(k p) o -> p k o", p=128).squeeze(-1))
        c_nb = wp.tile([128, NKT], BF16)
        nc.sync.dma_start(c_nb[:], norm_b[:].rearrange("(k p) o -> p k o", p=128).squeeze(-1))
        c_cb = wp.tile([128, NDT], F32)
        nc.sync.dma_start(c_cb[:], conv_b[:].rearrange("(k p) o -> p k o", p=128).squeeze(-1))
        c_dtb = wp.tile([128, NDT], F32)
        nc.sync.dma_start(c_dtb[:], dtproj_b[:].rearrange("(k p) o -> p k o", p=128).squeeze(-1))
        c_D = wp.tile([128, NDT], F32)
        nc.sync.dma_start(c_D[:], D_g[:].rearrange("(k p) o -> p k o", p=128).squeeze(-1))
        c_Alog = wp.tile([128, NDT * D_STATE], F32)
        nc.sync.dma_start(c_Alog[:].rearrange("p (k n) -> p k n", k=NDT),
                          A_log_g[:].rearrange("(k p) n -> p k n", p=128))
        ones1 = wp.tile([1, 128], F32)
        nc.vector.memset(ones1[:], 1.0)
        ones128 = wp.tile([128, 1], F32)
        nc.vector.memset(ones128[:], 1.0)
        eps_t = wp.tile([1, 1], F32)
        nc.vector.memset(eps_t[:], EPS)
        iden_bf = wp.tile([128, 128], BF16)
        make_identity(nc, iden_bf[:])

        if DEBUG:
            nc.sync.dma_start(dbg_ccb[:], c_cb[:])
            nc.sync.dma_start(dbg_wdiag[:], w_diag[:])

        # A = -exp(A_log): [128, NDT*16]
        c_A = wp.tile([128, NDT * D_STATE], F32)
        nc.scalar.activation(c_A[:], c_Alog[:], AF.Exp)
        nc.vector.tensor_scalar_mul(c_A[:], c_A[:], -1.0)

        # in_proj drain biases: nb @ W (folds norm_bias through in_proj)
        bias_sb = wp.tile([128, 2 * DG // 128], F32)
        with tc.tile_pool(name="biasps", bufs=2, space="PSUM") as bps:
            for mt in range(2 * DG // 128):
                bp = bps.tile([128, 1], F32, tag="bp", name="bp")
                for kt in range(NKT):
                    nc.tensor.matmul(
                        bp[:],
                        w_inproj[:, (kt * 2 * DG) + mt * 128:
                                 (kt * 2 * DG) + (mt + 1) * 128],
                        c_nb[:, kt:kt + 1],
                        start=(kt == 0), stop=(kt == NKT - 1))
                nc.scalar.activation(bias_sb[:, mt:mt + 1], bp[:], AF.Copy)

        # persistent activations (both batches)
        ap_ = ctx.enter_context(tc.tile_pool(name="acts", bufs=1))
        xi = [[ap_.tile([128, SEQ], BF16, tag=f"xi{b}{d}", name=f"xi{b}{d}") for d in range(NDT)]
              for b in range(BATCH)]
        siluz = [[ap_.tile([128, SEQ], BF16, tag=f"sz{b}{d}", name=f"sz{b}{d}") for d in range(NDT)]
                 for b in range(BATCH)]
        dt_t = [[ap_.tile([128, SEQ], BF16, tag=f"dt{b}{d}", name=f"dt{b}{d}") for d in range(NDT)]
                for b in range(BATCH)]

        # ---------- residual output (core's token quarter) ----------
        with tc.tile_pool(name="resq", bufs=2) as rp:
            for kt in range(NKT):
                th = rp.tile([128, QTOK], F32, tag="th", name="th")
                nc.sync.dma_start(th[:], hid_q[kt * 128:(kt + 1) * 128, :])
                tr = rp.tile([128, QTOK], F32, tag="tr", name="tr")
                nc.sync.dma_start(tr[:], res_q_in[kt * 128:(kt + 1) * 128, :])
                ts_ = rp.tile([128, QTOK], F32, tag="ts", name="ts")
                nc.gpsimd.tensor_add(ts_[:], th[:], tr[:])
                nc.sync.dma_start(res_q[kt * 128:(kt + 1) * 128, :], ts_[:])

        # ---------- phases A-D per batch ----------
        xi_pre = [ap_.tile([128, SEQ], BF16, tag=f"xp{d}", name=f"xp{d}") for d in range(NDT)]

        for b in range(BATCH):
            with tc.tile_pool(name=f"norm{b}", bufs=1) as np_, \
                 tc.tile_pool(name=f"normps{b}", bufs=1, space="PSUM") as nps, \
                 tc.tile_pool(name=f"mmps{b}", bufs=4, space="PSUM") as mps:
                for lh in range(2):
                    sl = slice(lh * LH, (lh + 1) * LH)
                    res_t = [np_.tile([128, LH], BF16, tag=f"res{k}", name=f"res{k}") for k in range(NKT)]
                    ssq = nps.tile([1, LH], F32, tag="ssq", name="ssq")
                    for kt in range(NKT):
                        th = np_.tile([128, LH], F32, tag="th", name="th")
                        nc.sync.dma_start(th[:], hid_T[b, kt * 128:(kt + 1) * 128, sl])
                        tr = np_.tile([128, LH], F32, tag="tr", name="tr")
                        nc.sync.dma_start(tr[:], res_T[b, kt * 128:(kt + 1) * 128, sl])
                        nc.vector.tensor_add(res_t[kt][:], th[:], tr[:])
                        sq = np_.tile([128, LH], F32, tag="sq", name="sq")
                        nc.scalar.activation(sq[:], res_t[kt][:], AF.Square)
                        for lc in range(LH // 512):
                            nc.tensor.matmul(ssq[:, lc * 512:(lc + 1) * 512],
                                             ones128[:],
                                             sq[:, lc * 512:(lc + 1) * 512],
                                             start=(kt == 0), stop=(kt == NKT - 1))
                    # rstd = exp(-0.5*ln(mean + eps))  (sqrt/rsqrt not in act tables)
                    lnv = np_.tile([1, LH], F32, tag="lnv", name="lnv")
                    nc.scalar.activation(lnv[:], ssq[:], AF.Ln, bias=eps_t[:],
                                         scale=1.0 / DIM)
                    rstd = np_.tile([1, LH], F32, tag="rstd", name="rstd")
                    nc.scalar.activation(rstd[:], lnv[:], AF.Exp, scale=-0.5)
                    rstd_rep = nps.tile([128, LH], F32, tag="rrep", name="rrep")
                    for lc in range(LH // 512):
                        nc.tensor.matmul(rstd_rep[:, lc * 512:(lc + 1) * 512],
                                         ones1[:], rstd[:, lc * 512:(lc + 1) * 512],
                                         start=True, stop=True)
                    rrep_sb = np_.tile([128, LH], BF16, tag="rrepsb", name="rrepsb")
                    nc.scalar.activation(rrep_sb[:], rstd_rep[:], AF.Copy)
                    x_t = []
                    for kt in range(NKT):
                        xx = np_.tile([128, LH], BF16, tag=f"x{kt}", name=f"x{kt}")
                        nc.vector.scalar_tensor_tensor(
                            xx[:], res_t[kt][:], c_nw[:, kt:kt + 1], rrep_sb[:],
                            OP.mult, OP.mult)
                        x_t.append(xx)
                    # in_proj for this L-half
                    for mt in range(2 * DG // 128):       # 4 m-tiles (2 xi + 2 z)
                        for lc in range(LH // 512):
                            pt = mps.tile([128, 512], F32, tag="mm", name="mm")
                            for kt in range(NKT):
                                nc.tensor.matmul(
                                    pt[:],
                                    w_inproj[:, (kt * 2 * DG) + mt * 128:
                                             (kt * 2 * DG) + (mt + 1) * 128],
                                    x_t[kt][:, lc * 512:(lc + 1) * 512],
                                    start=(kt == 0), stop=(kt == NKT - 1))
                            col = slice(lh * LH + lc * 512, lh * LH + (lc + 1) * 512)
                            if mt < NDT:
                                nc.scalar.activation(xi_pre[mt][:, col], pt[:],
                                                     AF.Identity,
                                                     bias=bias_sb[:, mt:mt + 1])
                            else:
                                nc.scalar.activation(siluz[b][mt - NDT][:, col], pt[:],
                                                     AF.Silu,
                                                     bias=bias_sb[:, mt:mt + 1])

            # ---------- conv (diag matmul) + silu, x_proj partial ----------
            with tc.tile_pool(name=f"cps{b}", bufs=4, space="PSUM") as cps, \
                 tc.tile_pool(name=f"cv{b}", bufs=2) as cvp:
                for d in range(NDT):
                    for lc in range(SEQ // 512):
                        pt = cps.tile([128, 512], F32, tag="conv", name="conv")
                        base = lc * 512
                        for j in range(D_CONV):
                            shift = D_CONV - 1 - j       # input col = out col - shift
                            lo, hi = base - shift, base + 512 - shift
                            olo = 0
                            if lo < 0:
                                olo, lo = -lo, 0
                            nc.tensor.matmul(
                                pt[:, olo:512],
                                w_diag[:, (j * NDT + d) * 128:(j * NDT + d + 1) * 128],
                                xi_pre[d][:, lo:hi],
                                start=(j == 0), stop=(j == D_CONV - 1),
                                skip_group_check=True)
                        nc.scalar.activation(xi[b][d][:, base:base + 512], pt[:],
                                             AF.Silu, bias=c_cb[:, d:d + 1])
                if DEBUG and b == 0:
                    for d in range(NDT):
                        nc.sync.dma_start(dbg_xipre[d * 128:(d + 1) * 128, :],
                                          xi_pre[d][:])
                # x_proj partial: [96, SEQ] = xproj_wT.T @ xi
                xdbl_sb = cvp.tile([NX, SEQ], BF16, tag="xdbl", name="xdbl")
                for lc in range(SEQ // 512):
                    pt = cps.tile([NX, 512], F32, tag="xproj", name="xproj")
                    for d in range(NDT):
                        nc.tensor.matmul(pt[:], w_xproj[:, d * NX:(d + 1) * NX],
                                         xi[b][d][:, lc * 512:(lc + 1) * 512],
                                         start=(d == 0), stop=(d == NDT - 1))
                    nc.scalar.activation(xdbl_sb[:, lc * 512:(lc + 1) * 512], pt[:],
                                         AF.Copy)
                if DEBUG and b == 0:
                    for d in range(NDT):
                        nc.sync.dma_start(dbg_xi[d * 128:(d + 1) * 128, :], xi[b][d][:])
                        nc.sync.dma_start(dbg_sz[d * 128:(d + 1) * 128, :], siluz[b][d][:])
                nc.sync.dma_start(ar_in[b][:], xdbl_sb[:])

            if SIM_NO_COLLECTIVES:
                nc.sync.dma_start(ar_out[b][:], ar_in[b][:])
            else:
                nc.gpsimd.collective_compute(
                    "AllReduce", OP.add, ins=[ar_in[b][:]], outs=[ar_out[b][:]],
                    replica_groups=GROUPS)

        # ---------- post-AR: dt_proj ----------
        du = [[ap_.tile([128, SEQ], BF16, tag=f"du{b}{d}", name=f"du{b}{d}") for d in range(NDT)]
              for b in range(BATCH)]
        for b in range(BATCH):
            with tc.tile_pool(name=f"dt{b}", bufs=2) as dp, \
                 tc.tile_pool(name=f"dtps{b}", bufs=4, space="PSUM") as dps:
                dtlow = dp.tile([DT_RANK, SEQ], BF16, tag="dtlow", name="dtlow")
                nc.sync.dma_start(dtlow[:], ar_out[b][0:DT_RANK, :])
                for d in range(NDT):
                    for lc in range(SEQ // 512):
                        pt = dps.tile([128, 512], F32, tag="dtmm", name="dtmm")
                        nc.tensor.matmul(pt[:], w_dtproj[:, d * 128:(d + 1) * 128],
                                         dtlow[:, lc * 512:(lc + 1) * 512],
                                         start=True, stop=True)
                        et = dp.tile([128, 512], F32, tag="spexp", name="spexp")
                        nc.scalar.activation(et[:], pt[:], AF.Exp,
                                             bias=c_dtb[:, d:d + 1])
                        nc.scalar.activation(dt_t[b][d][:, lc * 512:(lc + 1) * 512],
                                             et[:], AF.Ln, bias=ones128[:, 0:1])
                    nc.vector.tensor_tensor(du[b][d][:], dt_t[b][d][:], xi[b][d][:],
                                            OP.mult)
                    if DEBUG and b == 0:
                        nc.sync.dma_start(dbg_dt[d * 128:(d + 1) * 128, :], dt_t[b][d][:])
                if DEBUG and b == 0:
                    nc.sync.dma_start(dbg_xdbl[:], ar_out[b][:])

        # ---------- scan ----------
        yg = [[ap_.tile([128, SEQ], BF16, tag=f"du{b}{d}", name=f"du{b}{d}") for d in range(NDT)]
              for b in range(BATCH)]
        with tc.tile_pool(name="scanps", bufs=1, space="PSUM") as sps, \
             tc.tile_pool(name="scan", bufs=2) as sp:
            for b in range(BATCH):
                y_acc = [sps.tile([128, SEQ], F32, tag=f"yacc{d}", name=f"yacc{d}") for d in range(NDT)]
                hc_prev = [None] * NDT
                for n in range(D_STATE):
                    b_rep = sp.tile([128, SEQ], BF16, tag="brep", name="brep")
                    nc.sync.dma_start(
                        b_rep[:],
                        ar_out[b][DT_RANK + n:DT_RANK + n + 1, :].to_broadcast((128, SEQ)))
                    c_rep = sp.tile([128, SEQ], BF16, tag="crep", name="crep")
                    nc.sync.dma_start(
                        c_rep[:],
                        ar_out[b][DT_RANK + D_STATE + n:DT_RANK + D_STATE + n + 1, :]
                        .to_broadcast((128, SEQ)))
                    for d in range(NDT):
                        dA = sp.tile([128, SEQ], F32, tag="dA", name="dA")
                        nc.scalar.activation(dA[:], dt_t[b][d][:], AF.Exp,
                                             scale=c_A[:, d * D_STATE + n:
                                                       d * D_STATE + n + 1])
                        dBu = sp.tile([128, SEQ], BF16, tag="dBu", name="dBu")
                        nc.vector.tensor_tensor(dBu[:], du[b][d][:], b_rep[:], OP.mult)
                        h = sp.tile([128, SEQ], BF16, tag="h", name="h")
                        nc.vector.tensor_tensor_scan(h[:], dA[:], dBu[:], 0.0,
                                                     OP.mult, OP.add)
                        hC = sp.tile([128, SEQ], BF16, tag=f"hC{n % 2}",
                                     name=f"hC{n % 2}")
                        nc.vector.tensor_tensor(hC[:], h[:], c_rep[:], OP.mult)
                        if n % 2 == 0:
                            hc_prev[d] = hC
                        else:
                            hcp = sp.tile([128, SEQ], BF16, tag="hcp", name="hcp")
                            nc.vector.tensor_tensor(hcp[:], hc_prev[d][:], hC[:],
                                                    OP.add)
                            for lc in range(SEQ // 512):
                                nc.tensor.matmul(
                                    y_acc[d][:, lc * 512:(lc + 1) * 512], iden_bf[:],
                                    hcp[:, lc * 512:(lc + 1) * 512],
                                    start=(n == 1), stop=(n == D_STATE - 1),
                                    skip_group_check=True)
                # drain + gate:  yg = (y + D*u) * silu(z)
                for d in range(NDT):
                    t1 = sp.tile([128, SEQ], BF16, tag="t1", name="t1")
                    nc.vector.scalar_tensor_tensor(t1[:], xi[b][d][:], c_D[:, d:d + 1],
                                                   y_acc[d][:], OP.mult, OP.add)
                    nc.vector.tensor_tensor(yg[b][d][:], t1[:], siluz[b][d][:], OP.mult)
                    if DEBUG and b == 0:
                        nc.sync.dma_start(dbg_yg[d * 128:(d + 1) * 128, :], yg[b][d][:])

        # ---------- stage A2A ----------
        for b in range(BATCH):
            for d in range(NDT):
                for q in range(4):
                    s = b * 4 + q
                    nc.sync.dma_start(
                        a2a_in[s, d * 128:(d + 1) * 128, :],
                        yg[b][d][:, q * QTOK:(q + 1) * QTOK])
        if SIM_NO_COLLECTIVES:
            nc.sync.dma_start(a2a_out[:], a2a_in[:])
        else:
            nc.gpsimd.collective_compute("AllToAll", OP.bypass, ins=[a2a_in[:]],
                                         outs=[a2a_out[:]], replica_groups=GROUPS)

        # ---------- out_proj for the core's token quarter ----------
        with tc.tile_pool(name="oproj", bufs=2) as op_, \
             tc.tile_pool(name="ops", bufs=4, space="PSUM") as ops:
            w_outproj = op_.tile([128, (D_INNER // 128) * DIM], BF16, tag="wout", name="wout")
            nc.sync.dma_start(w_outproj[:].rearrange("p (k m) -> p k m", k=D_INNER // 128),
                              outproj_wT[:].rearrange("(k p) m -> p k m", p=128))
            yf = []
            for kt in range(D_INNER // 128):
                t = op_.tile([128, QTOK], BF16, tag=f"yf{kt}", name=f"yf{kt}")
                nc.sync.dma_start(t[:], a2a_out[:].rearrange("s d q -> (s d) q")
                                  [kt * 128:(kt + 1) * 128, :])
                yf.append(t)
            for mt in range(QTOK // 128):
                for nck in range(DIM // 512):
                    pt = ops.tile([128, 512], F32, tag="omm", name="omm")
                    for kt in range(D_INNER // 128):
                        nc.tensor.matmul(
                            pt[:], yf[kt][:, mt * 128:(mt + 1) * 128],
                            w_outproj[:, kt * DIM + nck * 512:
                                      kt * DIM + (nck + 1) * 512],
                            start=(kt == 0), stop=(kt == D_INNER // 128 - 1))
                    ot = op_.tile([128, 512], F32, tag="osb", name="osb")
                    nc.scalar.activation(ot[:], pt[:], AF.Copy)
                    nc.sync.dma_start(
                        out_q[mt * 128:(mt + 1) * 128, nck * 512:(nck + 1) * 512],
                        ot[:])

    nc.compile()
    _cache["nc"] = nc
    return nc


def _get_runner():
    """Cached shard_map jit over the bass custom call (adapted from
    bass2jax.run_bass_via_pjrt, which rebuilds its jit on every invocation)."""
    if "runner" in _cache:
        return _cache["runner"]
    nc = _build()

    import jax
    import concourse.bass2jax as b2j
    from concourse.bass2jax import _bass_exec_p, partition_id_tensor
    from jax.sharding import Mesh, PartitionSpec
    from jax.experimental.shard_map import shard_map

    b2j.install_neuronx_cc_hook()

    partition_name = nc.partition_id_tensor.name if nc.partition_id_tensor else None
    in_names, out_names, out_avals, zero_shapes = [], [], [], []
    for alloc in nc.m.functions[0].allocations:
        if not isinstance(alloc, mybir.MemoryLocationSet):
            continue
        name = alloc.memorylocations[0].name
        if alloc.kind == "ExternalInput":
            if name != partition_name:
                in_names.append(name)
        elif alloc.kind == "ExternalOutput":
            shape = tuple(alloc.tensor_shape)
            dtype = mybir.dt.np(alloc.dtype)
            out_names.append(name)
            out_avals.append(jax.core.ShapedArray(shape, dtype))
            zero_shapes.append((shape, dtype))
    n_params = len(in_names)
    n_outs = len(out_avals)
    all_in_names = list(in_names) + list(out_names)
    if partition_name is not None:
        all_in_names.append(partition_name)

    def _body(*args):
        operands = list(args)
        if partition_name is not None:
            operands.append(partition_id_tensor())
        return tuple(_bass_exec_p.bind(
            *operands, out_avals=tuple(out_avals),
            in_names=tuple(all_in_names), out_names=tuple(out_names),
            lowering_input_output_aliases=(), sim_require_finite=True,
            sim_require_nnan=True, nc=nc))

    devices = jax.devices()[:N_CORES]
    mesh = Mesh(np.asarray(devices), ("core",))
    donate = tuple(range(n_params, n_params + n_outs))
    sharded = jax.jit(
        shard_map(_body, mesh=mesh,
                  in_specs=(PartitionSpec("core"),) * (n_params + n_outs),
                  out_specs=(PartitionSpec("core"),) * n_outs,
                  check_rep=False),
        donate_argnums=donate, keep_unused=True)

    def run(in_maps):
        concat_in = [np.concatenate([np.asarray(in_maps[c][n]) for c in range(N_CORES)],
                                    axis=0) for n in in_names]
        concat_zeros = [np.zeros((N_CORES * s[0], *s[1:]), d) for s, d in zero_shapes]
        out_arrs = sharded(*concat_in, *concat_zeros)
        return [
            {n: np.asarray(out_arrs[i]).reshape(N_CORES, *out_avals[i].shape)[c]
             for i, n in enumerate(out_names)}
            for c in range(N_CORES)
        ]

    _cache["parts"] = (sharded, in_names, out_names, out_avals, zero_shapes, mesh)
    _cache["runner"] = run
    return run


def kernel(hidden_states, residual, norm_weight, norm_bias, in_proj_w, conv_w,
           conv_b, x_proj_w, dt_proj_w, dt_proj_b, A_log, D_param, out_proj_w):
    run = _get_runner()
    f32 = np.float32
    bf16 = None
    import ml_dtypes
    bf16 = ml_dtypes.bfloat16

    hid_T = np.ascontiguousarray(np.swapaxes(np.asarray(hidden_states, f32), 1, 2))
    res_T = np.ascontiguousarray(np.swapaxes(np.asarray(residual, f32), 1, 2))
    hid_flat = np.asarray(hidden_states, f32).reshape(BATCH * SEQ, DIM)
    res_flat = np.asarray(residual, f32).reshape(BATCH * SEQ, DIM)
    outproj_wT = np.ascontiguousarray(np.asarray(out_proj_w, f32).T).astype(bf16)

    in_maps = []
    for g in range(N_CORES):
        dg = slice(g * DG, (g + 1) * DG)
        w_x = np.asarray(in_proj_w[dg.start:dg.stop], f32)           # xi rows
        w_z = np.asarray(in_proj_w[D_INNER + dg.start:D_INNER + dg.stop], f32)
        inproj_wT = np.ascontiguousarray(np.concatenate([w_x, w_z], 0).T)
        cw = np.asarray(conv_w[dg], f32)                             # (256, 4)
        diag = np.zeros((D_CONV, NDT, 128, 128), f32)
        for j in range(D_CONV):
            for d in range(NDT):
                np.fill_diagonal(diag[j, d], cw[d * 128:(d + 1) * 128, j])
        qs = slice(g * QTOK, (g + 1) * QTOK)
        in_maps.append({
            "hid_T": hid_T,
            "res_T": res_T,
            "hid_q": np.ascontiguousarray(hid_flat[qs].T),
            "res_q_in": np.ascontiguousarray(res_flat[qs].T),
            "norm_w": np.asarray(norm_weight, f32).reshape(DIM, 1),
            "norm_b": np.asarray(norm_bias, f32).reshape(DIM, 1).astype(bf16),
            "inproj_wT": inproj_wT.astype(bf16),
            "conv_diag": diag.reshape(D_CONV * NDT * 128, 128).astype(bf16),
            "conv_b": np.asarray(conv_b[dg], f32).reshape(DG, 1),
            "xproj_wT": np.ascontiguousarray(np.asarray(x_proj_w, f32)[:, dg].T).astype(bf16),
            "dtproj_wT": np.ascontiguousarray(np.asarray(dt_proj_w, f32)[dg].T).astype(bf16),
            "dtproj_b": np.asarray(dt_proj_b[dg], f32).reshape(DG, 1),
            "A_log_g": np.asarray(A_log[dg], f32),
            "D_g": np.asarray(D_param[dg], f32).reshape(DG, 1),
            "outproj_wT": outproj_wT,
        })

    results = run(in_maps)

    out_flat = np.empty((BATCH * SEQ, DIM), f32)
    resid_flat = np.empty((BATCH * SEQ, DIM), f32)
    for g in range(N_CORES):
        qs = slice(g * QTOK, (g + 1) * QTOK)
        out_flat[qs] = results[g]["out_q"]
        resid_flat[qs] = results[g]["res_q"].T
    return (out_flat.reshape(BATCH, SEQ, DIM),
            resid_flat.reshape(BATCH, SEQ, DIM))
